# revision 26
# baseline (speedup 1.0000x reference)
"""MoE (top-1, E=8) TRN2 kernel — H-sharded merged-weight, fp8 hybrid.

out = x @ (Ws + We[e]).T + (bs + be[e])   (top-1 partition => one matmul)

v4: K-blocks 0,1 (256 of 1024) are computed in fp8e4 with a single
DoubleRow matmul (2x PE rate); blocks 2..7 stay fp16.  Scales are split
s_x = 1/8 on x and s_w = 8 on w so the fp8 product lands in PSUM at
scale 1 and accumulates directly with the fp16 blocks.  Measured
rel err 0.0160 (gate 2e-2).  Per tile: 1 DR MM + 6 fp16 MMs vs 8.
"""

import sys

sys.path.insert(0, "/opt/trn_rl_repo")

import numpy as np
import ml_dtypes

import concourse.bass as bass
import concourse.mybir as mybir
from concourse.tile import TileContext

N, D, H, E = 16384, 1024, 4096, 8
N_CORES = 8
KC = D // 128
HS = H // N_CORES      # 512: per-core H slice

KF8 = 2                # leading k-blocks in fp8 (one DoubleRow matmul)
KF16 = KC - KF8
S_W = 8.0              # fp8 weight scale; x gets 1/S_W
F8NP = mybir.dt.np(mybir.dt.float8e4)

F16 = mybir.dt.float16
F32 = mybir.dt.float32
F8 = mybir.dt.float8e4
DR = mybir.MatmulPerfMode.DoubleRow

MAX_WAITS = 1
N_DUMMY = 8            # N=512 warmup matmuls at COLD cadence (~427ns)
                       # exactly fill the 3.4us HAM busy window from
                       # ~7.3us, ending warm as tile 0's data lands


def split_long_waits(nc, max_w: int = MAX_WAITS):
    """walrus TPB_CTRL codegen rejects instructions with multiple sync
    waits; move excess waits onto same-engine NoOps."""
    n_fix = 0
    for f in nc.m.functions:
        for bb in f.blocks:
            insts = bb.instructions
            new_list = []
            changed = False
            for inst in insts:
                si = inst.sync_info
                if si is not None and len(si.on_wait) > max_w:
                    w = list(si.on_wait)
                    k = 0
                    while len(w) > max_w:
                        chunk, w = w[:max_w], w[max_w:]
                        nop = mybir.InstNoOp(
                            name=f"{inst.name}_waitsplit_{k}",
                            engine=inst.engine,
                            sync_info=mybir.SyncInfo(on_wait=chunk, on_update=[]),
                            bass_nofuse=True,
                        )
                        new_list.append(nop)
                        k += 1
                    inst.sync_info = mybir.SyncInfo(
                        on_wait=w, on_update=list(si.on_update)
                    )
                    n_fix += 1
                    changed = True
                new_list.append(inst)
            if changed:
                bb.instructions = new_list
    return n_fix


# ----------------------------------------------------------------------------
# device program (static schedule = expert id per token tile)
# ----------------------------------------------------------------------------


def build_program(sched: tuple, fix_waits: bool = True):
    nt = len(sched)
    nc = bass.Bass()

    w_d = nc.declare_dram_parameter("w16", [E, 128, KF16 * HS], F16, isOutput=False)
    w8_d = nc.declare_dram_parameter("w8", [E, 128, KF8 * HS], F8, isOutput=False)
    b_d = nc.declare_dram_parameter("b16", [128, E * HS], F16, isOutput=False)
    xg_d = nc.declare_dram_parameter(
        "xg16", [nt, 128, KF16 * 128], F16, isOutput=False
    )
    xg8_d = nc.declare_dram_parameter(
        "xg8", [nt, 128, KF8 * 128], F8, isOutput=False
    )
    out_d = nc.declare_dram_parameter("out", [nt * 128, HS], F16, isOutput=True)

    first_tile = {}
    for t, e in enumerate(sched):
        if e not in first_tile:
            first_tile[e] = t
    experts_used = sorted(first_tile, key=first_tile.get)

    # JIT weight/bias-chunk schedule: chunks ride the SCALAR queue
    # (HWDGE — no Q7 descriptor-emission cost), paced by the out-store
    # FIFO ahead of them.  Per expert: one 128 KB fp8 chunk, two 384 KB
    # fp16 half-chunks, one 128 KB bias chunk.  Slots >= 3 keep the
    # early-start window free for tile 0's critical loads.
    load_at = {}
    for e in experts_used[1:]:
        f = first_tile[e]
        for j, lead in ((0, 30), (1, 28), (2, 24), (3, 16)):
            slot = min(f - 1, max(3, f - lead))
            load_at.setdefault(max(0, slot), []).append((e, j))

    with TileContext(nc) as tc:
        with (
            tc.tile_pool(name="wres", bufs=1) as wpool,
            tc.tile_pool(name="xstream", bufs=9) as xpool,
            tc.tile_pool(name="ostage", bufs=6) as opool,
            tc.tile_pool(name="ps", bufs=6, space="PSUM") as pspool,
            tc.tile_pool(name="psdmy", bufs=1, space="PSUM") as dmypool,
        ):
            w = wpool.tile([128, E, KF16 * HS], F16, tag="w")
            w8 = wpool.tile([128, E, KF8, HS], F8, tag="w8")
            b = wpool.tile([128, E * HS], F16, tag="b")

            # PE warmup: short matmuls on a zeroed tile, result never
            # read; fills the gap between PE-queue availability (~7.3us,
            # after the framework preamble) and the first real matmul's
            # DMA, starting the HAM clock-gate warmup early.
            dmy = wpool.tile([128, 512], F16, tag="dmy")
            dps = dmypool.tile([128, 512], F32, tag="dps")
            nc.gpsimd.memset(dmy[:, :], 0.0)  # gpsimd is free ~0.6us before DVE
            for _ in range(N_DUMMY):
                nc.tensor.matmul(
                    dps[:, :],
                    lhsT=dmy[:, 0:128],
                    rhs=dmy[:, :],
                    start=True,
                    stop=True,
                )

            # Startup, ordered for tile 0's critical path under the
            # ~358 GB/s HBM cap shared round-robin across active
            # queues: sync carries xt8_0 + w8[e0] + w16h0 (DR matmul
            # first), scalar carries xt0 + w16h1 + bias[e0], gpsimd
            # carries x1.  Everything else (later x tiles, JIT chunks)
            # is paced behind these.  Steady state: even x tiles on
            # sync (HWDGE), odd on gpsimd (SWDGE); JIT chunks on the
            # scalar queue, paced by the out-store FIFO.
            e0 = sched[0]
            HALF = (KF16 // 2) * HS
            xts = {}
            xt0_8 = xpool.tile([128, KF8, 128], F8, tag="xt8", name="xt8_0")
            xt0 = xpool.tile([128, KF16 * 128], F16, tag="xt", name="xt0")
            # k-granular startup so each matmul of tile 0 gates on the
            # smallest possible transfer: sync carries the DR operands
            # + first fp16 k-block; scalar carries xt0 + the rest.
            nc.sync.dma_start(out=xt0_8[:, :, :], in_=xg8_d[0, :, :])
            nc.sync.dma_start(out=w8[:, e0, :, :], in_=w8_d[e0, :, :])
            nc.scalar.dma_start(out=xt0[:, :], in_=xg_d[0, :, :])
            nc.sync.dma_start(out=w[:, e0, 0:HS], in_=w_d[e0, :, 0:HS])
            nc.scalar.dma_start(out=w[:, e0, HS:HALF], in_=w_d[e0, :, HS:HALF])
            nc.scalar.dma_start(
                out=b[:, e0 * HS : (e0 + 1) * HS], in_=b_d[:, e0 * HS : (e0 + 1) * HS]
            )
            xts[0] = (xt0, xt0_8)
            if nt > 1:
                xt1_8 = xpool.tile([128, KF8, 128], F8, tag="xt8", name="xt8_1")
                xt1 = xpool.tile([128, KF16 * 128], F16, tag="xt", name="xt1")
                nc.gpsimd.dma_start(out=xt1_8[:, :, :], in_=xg8_d[1, :, :])
                nc.gpsimd.dma_start(out=xt1[:, :], in_=xg_d[1, :, :])
                xts[1] = (xt1, xt1_8)
            # k5-7 weights ride the otherwise-idle gpsimd queue so the
            # scalar chain (xt0 -> wk34 -> b0) shortens by 384 KB
            nc.gpsimd.dma_start(
                out=w[:, e0, HALF : KF16 * HS], in_=w_d[e0, :, HALF : KF16 * HS]
            )

            # Pace the JIT chunks and the x prefetch with scheduler
            # time-waits (tile_wait_until) — the Tile scheduler orders
            # by dependency+priority, NOT emission order, so without
            # these everything with free deps fires at t=0 and starves
            # tile 0's critical loads on the shared HBM pipe.
            SIM_TILE = 0.0014  # ~sim-time per token tile, in ms

            def emit_jit(t):
                for el, j in load_at.get(t, ()):
                    with tc.tile_wait_until(t * SIM_TILE):
                        if j == 0:
                            nc.scalar.dma_start(
                                out=w8[:, el, :, :], in_=w8_d[el, :, :]
                            )
                        elif j == 1:
                            nc.scalar.dma_start(
                                out=w[:, el, 0:HALF], in_=w_d[el, :, 0:HALF]
                            )
                        elif j == 2:
                            nc.scalar.dma_start(
                                out=w[:, el, HALF : KF16 * HS],
                                in_=w_d[el, :, HALF : KF16 * HS],
                            )
                        else:
                            nc.scalar.dma_start(
                                out=b[:, el * HS : (el + 1) * HS],
                                in_=b_d[:, el * HS : (el + 1) * HS],
                            )

            for t in range(nt):
                e = sched[t]
                emit_jit(t)
                if t in xts:
                    xt, xt8 = xts[t]
                else:
                    xt8 = xpool.tile([128, KF8, 128], F8, tag="xt8")
                    xt = xpool.tile([128, KF16 * 128], F16, tag="xt")
                    q = nc.sync if t % 2 == 0 else nc.gpsimd
                    xw = max((t - 8) * SIM_TILE, (t - 2) * 0.0008)
                    with tc.tile_wait_until(xw):
                        q.dma_start(out=xt8[:, :, :], in_=xg8_d[t, :, :])
                        q.dma_start(out=xt[:, :], in_=xg_d[t, :, :])
                ot = opool.tile([128, HS], F16, tag="ot")
                ps = pspool.tile([128, HS], F32, tag="ps")
                nc.tensor.matmul(
                    ps[:, :],
                    lhsT=xt8[:, :, :],
                    rhs=w8[:, e, :, :],
                    perf_mode=DR,
                    start=True,
                    stop=False,
                )
                for k in range(KF16):
                    nc.tensor.matmul(
                        ps[:, :],
                        lhsT=xt[:, k * 128 : (k + 1) * 128],
                        rhs=w[:, e, k * HS : (k + 1) * HS],
                        start=False,
                        stop=(k == KF16 - 1),
                    )
                if t == nt - 1:
                    # tail: split the last bias-add + store in halves on
                    # two HWDGE queues so the HBM completion receipts
                    # overlap instead of serializing after the last MM.
                    HH = HS // 2
                    for qi in range(2):
                        lo, hi = qi * HH, (qi + 1) * HH
                        nc.vector.tensor_add(
                            out=ot[:, lo:hi],
                            in0=ps[:, lo:hi],
                            in1=b[:, e * HS + lo : e * HS + hi],
                        )
                        qq = nc.sync if qi % 2 == 0 else nc.scalar
                        qq.dma_start(
                            out=out_d[t * 128 : (t + 1) * 128, lo:hi],
                            in_=ot[:, lo:hi],
                        )
                else:
                    nc.vector.tensor_add(
                        out=ot[:, :],
                        in0=ps[:, :],
                        in1=b[:, e * HS : (e + 1) * HS],
                    )
                    nc.scalar.dma_start(
                        out=out_d[t * 128 : (t + 1) * 128, :], in_=ot[:, :]
                    )

    if fix_waits:
        split_long_waits(nc)
    return nc


# ----------------------------------------------------------------------------
# host-side routing / packing / scatter
# ----------------------------------------------------------------------------


def route(te):
    """-> (sched tuple, tokens [nt*128] with -1 pads)."""
    sched = []
    toks = []
    for e in range(E):
        ids = np.nonzero(te == e)[0]
        if len(ids) == 0:
            continue
        nt_e = int(np.ceil(len(ids) / 128))
        pad = np.full(nt_e * 128, -1, dtype=np.int64)
        pad[: len(ids)] = ids
        sched += [e] * nt_e
        toks.append(pad)
    return tuple(sched), np.concatenate(toks)


def _tile_x16(x16, tk):
    xt = x16[tk][:, KF8 * 128 :]  # [nt*128, 768]
    m = len(tk) // 128
    return np.ascontiguousarray(
        xt.reshape(m, 128, KF16, 128)
        .transpose(0, 3, 2, 1)
        .reshape(m, 128, KF16 * 128)
    )


def _tile_x8(x, tk):
    x8 = ((x[:, : KF8 * 128] * (1.0 / S_W)).astype(F8NP))[tk]  # [nt*128, 256]
    m = len(tk) // 128
    return np.ascontiguousarray(
        x8.reshape(m, 128, KF8, 128).transpose(0, 3, 2, 1).reshape(m, 128, KF8 * 128)
    )


def make_in_maps(x, Ws, bs, We, be, toks):
    x16 = x.astype(np.float16)
    tk = np.where(toks < 0, 0, toks)
    xg = _tile_x16(x16, tk)
    xg8 = _tile_x8(x, tk)
    in_maps = []
    for c in range(N_CORES):
        ws = np.empty((E, 128, KF16 * HS), dtype=np.float16)
        ws8 = np.empty((E, 128, KF8 * HS), dtype=F8NP)
        bias = np.empty((128, E * HS), dtype=np.float16)
        for e in range(E):
            WT = (Ws + We[e]).T[:, c * HS : (c + 1) * HS]  # [D, HS] fp32
            ws[e] = (
                WT[KF8 * 128 :]
                .reshape(KF16, 128, HS)
                .transpose(1, 0, 2)
                .reshape(128, KF16 * HS)
            ).astype(np.float16)
            ws8[e] = (
                (WT[: KF8 * 128] * S_W)
                .reshape(KF8, 128, HS)
                .transpose(1, 0, 2)
                .reshape(128, KF8 * HS)
            ).astype(F8NP)
            bias[:, e * HS : (e + 1) * HS] = (
                (bs + be[e])[c * HS : (c + 1) * HS].astype(np.float16)
            )
        in_maps.append({"w16": ws, "w8": ws8, "b16": bias, "xg16": xg, "xg8": xg8})
    return in_maps


def scatter_out(results, toks):
    out = np.empty((N, H), dtype=np.float32)
    valid = toks >= 0
    tv = toks[valid]
    for c in range(N_CORES):
        rows = results[c]["out"]  # [nt*128, HS] fp16
        out[tv, c * HS : (c + 1) * HS] = rows[valid].astype(np.float32)
    return out


# ----------------------------------------------------------------------------
# entry point
# ----------------------------------------------------------------------------

_PROGRAM_CACHE = {}


def _get_program(sched):
    if sched not in _PROGRAM_CACHE:
        _PROGRAM_CACHE[sched] = build_program(sched)
    return _PROGRAM_CACHE[sched]


def prepare(x, Ws, bs, We, be, Wr, br):
    x = np.asarray(x, dtype=np.float32)
    Ws = np.asarray(Ws, dtype=np.float32)
    bs = np.asarray(bs, dtype=np.float32)
    We = np.asarray(We, dtype=np.float32)
    be = np.asarray(be, dtype=np.float32)
    Wr = np.asarray(Wr, dtype=np.float32)
    br = np.asarray(br, dtype=np.float32)
    assert x.shape == (N, D)

    logits = x @ Wr.T + br
    te = np.argmax(logits, axis=-1)
    sched, toks = route(te)
    nc = _get_program(sched)
    in_maps = make_in_maps(x, Ws, bs, We, be, toks)
    return nc, in_maps, toks


def finish(results, toks):
    return scatter_out(results, toks)


def kernel(x, Ws, bs, We, be, Wr, br):
    from concourse.bass_utils import run_bass_kernel_spmd

    nc, in_maps, toks = prepare(x, Ws, bs, We, be, Wr, br)
    res = run_bass_kernel_spmd(nc, in_maps, list(range(N_CORES)))
    return finish(res.results, toks)


# revision 27
# speedup vs baseline: 1.1887x; 1.1887x over previous
"""MoE (top-1, E=8) TRN2 kernel — H-sharded merged-weight, fp8 hybrid.

out = x @ (Ws + We[e]).T + (bs + be[e])   (top-1 partition => one matmul)

v4: K-blocks 0,1 (256 of 1024) are computed in fp8e4 with a single
DoubleRow matmul (2x PE rate); blocks 2..7 stay fp16.  Scales are split
s_x = 1/8 on x and s_w = 8 on w so the fp8 product lands in PSUM at
scale 1 and accumulates directly with the fp16 blocks.  Measured
rel err 0.0160 (gate 2e-2).  Per tile: 1 DR MM + 6 fp16 MMs vs 8.
"""

import sys

sys.path.insert(0, "/opt/trn_rl_repo")

import numpy as np
import ml_dtypes

import concourse.bass as bass
import concourse.mybir as mybir
from concourse.tile import TileContext

N, D, H, E = 16384, 1024, 4096, 8
N_CORES = 8
KC = D // 128
HS = H // N_CORES      # 512: per-core H slice

KF8 = 2                # leading k-blocks in fp8 (one DoubleRow matmul)
KF16 = KC - KF8
S_W = 8.0              # fp8 weight scale; x gets 1/S_W
F8NP = mybir.dt.np(mybir.dt.float8e4)

F16 = mybir.dt.float16
F32 = mybir.dt.float32
F8 = mybir.dt.float8e4
DR = mybir.MatmulPerfMode.DoubleRow

MAX_WAITS = 1
N_DUMMY = 9            # N=512 warmup matmuls at COLD cadence (~427ns):
                       # 8 fill the 3.4us HAM busy window; chain ends
                       # ~11.5us, just as tile 0's data lands warm


def split_long_waits(nc, max_w: int = MAX_WAITS):
    """walrus TPB_CTRL codegen rejects instructions with multiple sync
    waits; move excess waits onto same-engine NoOps."""
    n_fix = 0
    for f in nc.m.functions:
        for bb in f.blocks:
            insts = bb.instructions
            new_list = []
            changed = False
            for inst in insts:
                si = inst.sync_info
                if si is not None and len(si.on_wait) > max_w:
                    w = list(si.on_wait)
                    k = 0
                    while len(w) > max_w:
                        chunk, w = w[:max_w], w[max_w:]
                        nop = mybir.InstNoOp(
                            name=f"{inst.name}_waitsplit_{k}",
                            engine=inst.engine,
                            sync_info=mybir.SyncInfo(on_wait=chunk, on_update=[]),
                            bass_nofuse=True,
                        )
                        new_list.append(nop)
                        k += 1
                    inst.sync_info = mybir.SyncInfo(
                        on_wait=w, on_update=list(si.on_update)
                    )
                    n_fix += 1
                    changed = True
                new_list.append(inst)
            if changed:
                bb.instructions = new_list
    return n_fix


# ----------------------------------------------------------------------------
# device program (static schedule = expert id per token tile)
# ----------------------------------------------------------------------------


def build_program(sched: tuple, fix_waits: bool = True):
    nt = len(sched)
    nc = bass.Bass()

    w_d = nc.declare_dram_parameter("w16", [E, 128, KF16 * HS], F16, isOutput=False)
    w8_d = nc.declare_dram_parameter("w8", [E, 128, KF8 * HS], F8, isOutput=False)
    b_d = nc.declare_dram_parameter("b16", [128, E * HS], F16, isOutput=False)
    xg_d = nc.declare_dram_parameter(
        "xg16", [nt, 128, KF16 * 128], F16, isOutput=False
    )
    xg8_d = nc.declare_dram_parameter(
        "xg8", [nt, 128, KF8 * 128], F8, isOutput=False
    )
    out_d = nc.declare_dram_parameter("out", [nt * 128, HS], F16, isOutput=True)

    first_tile = {}
    for t, e in enumerate(sched):
        if e not in first_tile:
            first_tile[e] = t
    experts_used = sorted(first_tile, key=first_tile.get)

    # JIT weight/bias-chunk schedule: chunks ride the SCALAR queue
    # (HWDGE — no Q7 descriptor-emission cost), paced by the out-store
    # FIFO ahead of them.  Per expert: one 128 KB fp8 chunk, two 384 KB
    # fp16 half-chunks, one 128 KB bias chunk.  Slots >= 3 keep the
    # early-start window free for tile 0's critical loads.
    load_at = {}
    for e in experts_used[1:]:
        f = first_tile[e]
        for j, lead in ((0, 30), (1, 28), (2, 24), (3, 16)):
            slot = min(f - 1, max(3, f - lead))
            load_at.setdefault(max(0, slot), []).append((e, j))

    with TileContext(nc) as tc:
        with (
            tc.tile_pool(name="wres", bufs=1) as wpool,
            tc.tile_pool(name="xstream", bufs=9) as xpool,
            tc.tile_pool(name="ostage", bufs=6) as opool,
            tc.tile_pool(name="ps", bufs=6, space="PSUM") as pspool,
            tc.tile_pool(name="psdmy", bufs=1, space="PSUM") as dmypool,
        ):
            w = wpool.tile([128, E, KF16 * HS], F16, tag="w")
            w8 = wpool.tile([128, E, KF8, HS], F8, tag="w8")
            b = wpool.tile([128, E * HS], F16, tag="b")

            # PE warmup: short matmuls on a zeroed tile, result never
            # read; fills the gap between PE-queue availability (~7.3us,
            # after the framework preamble) and the first real matmul's
            # DMA, starting the HAM clock-gate warmup early.
            dmy = wpool.tile([128, 512], F16, tag="dmy")
            dps = dmypool.tile([128, 512], F32, tag="dps")
            nc.vector.memset(dmy[:, :], 0.0)
            for _ in range(N_DUMMY):
                nc.tensor.matmul(
                    dps[:, :],
                    lhsT=dmy[:, 0:128],
                    rhs=dmy[:, :],
                    start=True,
                    stop=True,
                )

            # Startup, ordered for tile 0's critical path under the
            # ~358 GB/s HBM cap shared round-robin across active
            # queues: sync carries xt8_0 + w8[e0] + w16h0 (DR matmul
            # first), scalar carries xt0 + w16h1 + bias[e0], gpsimd
            # carries x1.  Everything else (later x tiles, JIT chunks)
            # is paced behind these.  Steady state: even x tiles on
            # sync (HWDGE), odd on gpsimd (SWDGE); JIT chunks on the
            # scalar queue, paced by the out-store FIFO.
            e0 = sched[0]
            HALF = (KF16 // 2) * HS
            xts = {}
            xt0_8 = xpool.tile([128, KF8, 128], F8, tag="xt8", name="xt8_0")
            xt0 = xpool.tile([128, KF16 * 128], F16, tag="xt", name="xt0")
            # k-granular startup so each matmul of tile 0 gates on the
            # smallest possible transfer: sync carries the DR operands
            # + first fp16 k-block; scalar carries xt0 + the rest.
            nc.sync.dma_start(out=xt0_8[:, :, :], in_=xg8_d[0, :, :])
            nc.sync.dma_start(out=w8[:, e0, :, :], in_=w8_d[e0, :, :])
            nc.scalar.dma_start(out=xt0[:, :], in_=xg_d[0, :, :])
            nc.sync.dma_start(out=w[:, e0, 0:HS], in_=w_d[e0, :, 0:HS])
            nc.scalar.dma_start(out=w[:, e0, HS:HALF], in_=w_d[e0, :, HS:HALF])
            nc.scalar.dma_start(
                out=b[:, e0 * HS : (e0 + 1) * HS], in_=b_d[:, e0 * HS : (e0 + 1) * HS]
            )
            xts[0] = (xt0, xt0_8)
            if nt > 1:
                xt1_8 = xpool.tile([128, KF8, 128], F8, tag="xt8", name="xt8_1")
                xt1 = xpool.tile([128, KF16 * 128], F16, tag="xt", name="xt1")
                nc.gpsimd.dma_start(out=xt1_8[:, :, :], in_=xg8_d[1, :, :])
                nc.gpsimd.dma_start(out=xt1[:, :], in_=xg_d[1, :, :])
                xts[1] = (xt1, xt1_8)
            # k5-7 weights ride the otherwise-idle gpsimd queue so the
            # scalar chain (xt0 -> wk34 -> b0) shortens by 384 KB
            nc.gpsimd.dma_start(
                out=w[:, e0, HALF : KF16 * HS], in_=w_d[e0, :, HALF : KF16 * HS]
            )

            # Pace the JIT chunks and the x prefetch with scheduler
            # time-waits (tile_wait_until) — the Tile scheduler orders
            # by dependency+priority, NOT emission order, so without
            # these everything with free deps fires at t=0 and starves
            # tile 0's critical loads on the shared HBM pipe.
            SIM_TILE = 0.0014  # ~sim-time per token tile, in ms

            def emit_jit(t):
                for el, j in load_at.get(t, ()):
                    with tc.tile_wait_until(t * SIM_TILE):
                        if j == 0:
                            nc.scalar.dma_start(
                                out=w8[:, el, :, :], in_=w8_d[el, :, :]
                            )
                        elif j == 1:
                            nc.scalar.dma_start(
                                out=w[:, el, 0:HALF], in_=w_d[el, :, 0:HALF]
                            )
                        elif j == 2:
                            nc.scalar.dma_start(
                                out=w[:, el, HALF : KF16 * HS],
                                in_=w_d[el, :, HALF : KF16 * HS],
                            )
                        else:
                            nc.scalar.dma_start(
                                out=b[:, el * HS : (el + 1) * HS],
                                in_=b_d[:, el * HS : (el + 1) * HS],
                            )

            for t in range(nt):
                e = sched[t]
                emit_jit(t)
                if t in xts:
                    xt, xt8 = xts[t]
                else:
                    xt8 = xpool.tile([128, KF8, 128], F8, tag="xt8")
                    xt = xpool.tile([128, KF16 * 128], F16, tag="xt")
                    q = nc.sync if t % 2 == 0 else nc.gpsimd
                    xw = max((t - 8) * SIM_TILE, (t - 2) * 0.0008)
                    with tc.tile_wait_until(xw):
                        q.dma_start(out=xt8[:, :, :], in_=xg8_d[t, :, :])
                        q.dma_start(out=xt[:, :], in_=xg_d[t, :, :])
                ot = opool.tile([128, HS], F16, tag="ot")
                ps = pspool.tile([128, HS], F32, tag="ps")
                nc.tensor.matmul(
                    ps[:, :],
                    lhsT=xt8[:, :, :],
                    rhs=w8[:, e, :, :],
                    perf_mode=DR,
                    start=True,
                    stop=False,
                )
                for k in range(KF16):
                    nc.tensor.matmul(
                        ps[:, :],
                        lhsT=xt[:, k * 128 : (k + 1) * 128],
                        rhs=w[:, e, k * HS : (k + 1) * HS],
                        start=False,
                        stop=(k == KF16 - 1),
                    )
                if t == nt - 1:
                    # tail: split the last bias-add + store in halves on
                    # two HWDGE queues so the HBM completion receipts
                    # overlap instead of serializing after the last MM.
                    HH = HS // 2
                    for qi in range(2):
                        lo, hi = qi * HH, (qi + 1) * HH
                        nc.vector.tensor_add(
                            out=ot[:, lo:hi],
                            in0=ps[:, lo:hi],
                            in1=b[:, e * HS + lo : e * HS + hi],
                        )
                        qq = nc.sync if qi % 2 == 0 else nc.scalar
                        qq.dma_start(
                            out=out_d[t * 128 : (t + 1) * 128, lo:hi],
                            in_=ot[:, lo:hi],
                        )
                else:
                    nc.vector.tensor_add(
                        out=ot[:, :],
                        in0=ps[:, :],
                        in1=b[:, e * HS : (e + 1) * HS],
                    )
                    nc.scalar.dma_start(
                        out=out_d[t * 128 : (t + 1) * 128, :], in_=ot[:, :]
                    )

    if fix_waits:
        split_long_waits(nc)
    return nc


# ----------------------------------------------------------------------------
# host-side routing / packing / scatter
# ----------------------------------------------------------------------------


def route(te):
    """-> (sched tuple, tokens [nt*128] with -1 pads)."""
    sched = []
    toks = []
    for e in range(E):
        ids = np.nonzero(te == e)[0]
        if len(ids) == 0:
            continue
        nt_e = int(np.ceil(len(ids) / 128))
        pad = np.full(nt_e * 128, -1, dtype=np.int64)
        pad[: len(ids)] = ids
        sched += [e] * nt_e
        toks.append(pad)
    return tuple(sched), np.concatenate(toks)


def _tile_x16(x16, tk):
    xt = x16[tk][:, KF8 * 128 :]  # [nt*128, 768]
    m = len(tk) // 128
    return np.ascontiguousarray(
        xt.reshape(m, 128, KF16, 128)
        .transpose(0, 3, 2, 1)
        .reshape(m, 128, KF16 * 128)
    )


def _tile_x8(x, tk):
    x8 = ((x[:, : KF8 * 128] * (1.0 / S_W)).astype(F8NP))[tk]  # [nt*128, 256]
    m = len(tk) // 128
    return np.ascontiguousarray(
        x8.reshape(m, 128, KF8, 128).transpose(0, 3, 2, 1).reshape(m, 128, KF8 * 128)
    )


def make_in_maps(x, Ws, bs, We, be, toks):
    x16 = x.astype(np.float16)
    tk = np.where(toks < 0, 0, toks)
    xg = _tile_x16(x16, tk)
    xg8 = _tile_x8(x, tk)
    in_maps = []
    for c in range(N_CORES):
        ws = np.empty((E, 128, KF16 * HS), dtype=np.float16)
        ws8 = np.empty((E, 128, KF8 * HS), dtype=F8NP)
        bias = np.empty((128, E * HS), dtype=np.float16)
        for e in range(E):
            WT = (Ws + We[e]).T[:, c * HS : (c + 1) * HS]  # [D, HS] fp32
            ws[e] = (
                WT[KF8 * 128 :]
                .reshape(KF16, 128, HS)
                .transpose(1, 0, 2)
                .reshape(128, KF16 * HS)
            ).astype(np.float16)
            ws8[e] = (
                (WT[: KF8 * 128] * S_W)
                .reshape(KF8, 128, HS)
                .transpose(1, 0, 2)
                .reshape(128, KF8 * HS)
            ).astype(F8NP)
            bias[:, e * HS : (e + 1) * HS] = (
                (bs + be[e])[c * HS : (c + 1) * HS].astype(np.float16)
            )
        in_maps.append({"w16": ws, "w8": ws8, "b16": bias, "xg16": xg, "xg8": xg8})
    return in_maps


def scatter_out(results, toks):
    out = np.empty((N, H), dtype=np.float32)
    valid = toks >= 0
    tv = toks[valid]
    for c in range(N_CORES):
        rows = results[c]["out"]  # [nt*128, HS] fp16
        out[tv, c * HS : (c + 1) * HS] = rows[valid].astype(np.float32)
    return out


# ----------------------------------------------------------------------------
# entry point
# ----------------------------------------------------------------------------

_PROGRAM_CACHE = {}


def _get_program(sched):
    if sched not in _PROGRAM_CACHE:
        _PROGRAM_CACHE[sched] = build_program(sched)
    return _PROGRAM_CACHE[sched]


def prepare(x, Ws, bs, We, be, Wr, br):
    x = np.asarray(x, dtype=np.float32)
    Ws = np.asarray(Ws, dtype=np.float32)
    bs = np.asarray(bs, dtype=np.float32)
    We = np.asarray(We, dtype=np.float32)
    be = np.asarray(be, dtype=np.float32)
    Wr = np.asarray(Wr, dtype=np.float32)
    br = np.asarray(br, dtype=np.float32)
    assert x.shape == (N, D)

    logits = x @ Wr.T + br
    te = np.argmax(logits, axis=-1)
    sched, toks = route(te)
    nc = _get_program(sched)
    in_maps = make_in_maps(x, Ws, bs, We, be, toks)
    return nc, in_maps, toks


def finish(results, toks):
    return scatter_out(results, toks)


def kernel(x, Ws, bs, We, be, Wr, br):
    from concourse.bass_utils import run_bass_kernel_spmd

    nc, in_maps, toks = prepare(x, Ws, bs, We, be, Wr, br)
    res = run_bass_kernel_spmd(nc, in_maps, list(range(N_CORES)))
    return finish(res.results, toks)
